# revision 26
# baseline (speedup 1.0000x reference)
"""Trainium2 Bass kernel for nn_MaxPool_730144440853.

Math (per batch b):
    d = einsum("czn,dc->dzn", x[b], W)
    scores[c, n] = sum_z x[b,c,z,n] * d[c,z,n]
    idx[c] = argmax_n scores[c, n]
    out[b, c, :] = x[b, c, :, idx[c]]

Sharding: data-parallel over batch B=8 across the 8 NeuronCores; W replicated.

Device pipeline (per core, fp16 inputs):
  - PE: d = W @ x per (n-tile, half, z), fp16 matmuls into fp32 PSUM, k-major
    so 3 consecutive matmuls share stationary weights.
  - Act: cast d PSUM fp32 -> SBUF fp16 (unlocks the DVE 2x 16-bit mode).
  - DVE (2x fp16): p = x * d, one fused strided-slab instruction for both
    z-adds, and a running elementwise fold m[q] = max_t s[t*512+q] across the
    16 n-tiles. Everything stays on DVE: cross-engine edges through Pool get
    mis-scheduled (the tile scheduler's GpSimd cost model is ~1.8x fast).
  - DVE tail: max8 + max_index over the folded [128, 512] max array only.

Device returns, per (b, c) row, the top-8 folded positions q_j. The true
argmax position n* = t*512+q satisfies q in {q_j} unless >=8 distinct folded
positions beat the true maximum within fp16 noise (~0.1% of sigma), which is
astronomically unlikely. Host expands the 8 q's to 8*16=128 candidate n's,
re-scores them exactly in float64 from the original fp32 inputs, and picks
the argmax (ties toward smallest n, matching jnp.argmax first-occurrence).
"""

import sys

sys.path.insert(0, "/opt/trn_rl_repo")

import numpy as np

B, C, Z, N = 8, 256, 3, 8192
H = C // 128  # partition halves (2)
T = 512  # n-tile width
NT = N // T
ZT = Z * T

_cache = {}


def _split_multiwait_bir(bir_json: bytes) -> bytes:
    """walrus in this toolchain rejects instructions carrying more than one
    semaphore wait ("Too many sync wait commands"). Rewrite the BIR so any
    instruction with >1 on_wait keeps only the last one; the others are
    hoisted into single-wait EventSemaphore instructions inserted just
    before it on the same engine (engine program order makes this
    equivalent)."""
    import json

    d = json.loads(bir_json)
    n_new = 0
    for fn in d.get("functions", []):
        for blk in fn.get("blocks", []):
            insts = blk.get("instructions", [])
            out = []
            for ins in insts:
                si = ins.get("sync_info")
                waits = si.get("on_wait") if si else None
                if waits and len(waits) > 1:
                    for w in waits[:-1]:
                        out.append(
                            {
                                "debug": ins.get("debug", 0),
                                "engine": ins["engine"],
                                "ins": [],
                                "name": f"{ins['name']}_hw{n_new}",
                                "opcode": "EventSemaphore",
                                "outs": [],
                                "sync_info": {"on_update": [], "on_wait": [w]},
                            }
                        )
                        n_new += 1
                    si["on_wait"] = [waits[-1]]
                out.append(ins)
            blk["instructions"] = out
    return json.dumps(d).encode()


def _apply_tile_patch():
    """Install the multi-wait splitter in front of walrus compilation."""
    from concourse import bass_utils, bass2jax

    if getattr(bass_utils, "_ant_split_multiwait", False):
        return

    orig = bass_utils.compile_bir_kernel

    def patched(bir_json, tmpdir, neff_name="file.neff"):
        return orig(_split_multiwait_bir(bir_json), tmpdir, neff_name=neff_name)

    bass_utils.compile_bir_kernel = patched
    bass2jax.compile_bir_kernel = patched
    bass_utils._ant_split_multiwait = True


def _build_nc():
    import concourse.bass as bass
    import concourse.mybir as mybir
    from concourse.tile import TileContext

    _apply_tile_patch()

    f16 = mybir.dt.float16
    f32 = mybir.dt.float32
    u32 = mybir.dt.uint32
    add = mybir.AluOpType.add
    mult = mybir.AluOpType.mult
    vmax = mybir.AluOpType.max

    nc = bass.Bass(target_bir_lowering=False)
    # x{k}[t] = fp16 tile [128, Z*T], channels k*128..k*128+127, n-tile t.
    x0 = nc.dram_tensor("x0", [NT, 128, ZT], f16, kind="ExternalInput")
    x1 = nc.dram_tensor("x1", [NT, 128, ZT], f16, kind="ExternalInput")
    # wt{k}[c_in - k*128, c_out] = W[c_out, c_in]; lhsT slices for the PE.
    wt0 = nc.dram_tensor("wt0", [128, C], f16, kind="ExternalInput")
    wt1 = nc.dram_tensor("wt1", [128, C], f16, kind="ExternalInput")
    v8 = nc.dram_tensor("v8", [H, 128, 8], f16, kind="ExternalOutput")
    i8 = nc.dram_tensor("i8", [H, 128, 8], u32, kind="ExternalOutput")

    with TileContext(nc) as tc:
        with (
            tc.tile_pool(name="wts", bufs=1) as wpool,
            tc.tile_pool(name="xin", bufs=6) as xpool,
            tc.tile_pool(name="dcast", bufs=6) as cpool,
            tc.tile_pool(name="prod", bufs=6) as ppool,
            tc.tile_pool(name="fold", bufs=2) as mpool,
            tc.tile_pool(name="psum", bufs=2, space="PSUM") as dpool,
            tc.tile_pool(name="outs", bufs=1) as opool,
        ):
            # First x chunk is dispatched before the W loads: W is only 64KB
            # and lands quickly, while the first matmul's x slice is the long
            # pole of the pipeline head.
            x0_first = xpool.tile([128, Z, T], f16, tag="x0", name="xt0_first")
            for half in range(2):
                nc.sync.dma_start(
                    out=x0_first[:, 0, half * (T // 2) : (half + 1) * (T // 2)],
                    in_=x0[0, :, half * (T // 2) : (half + 1) * (T // 2)],
                )

            wt_sb = []
            for k, wt in enumerate((wt0, wt1)):
                w = wpool.tile([128, C], f16, tag=f"wt{k}", name=f"wt_sb{k}")
                nc.sync.dma_start(out=w[:], in_=wt[:])
                wt_sb.append(w)

            # running folded max per half, ping-pong buffers via the pool
            # (memsets on the otherwise-idle Pool engine, off the DVE queue)
            m_prev = []
            for h in range(H):
                m0 = mpool.tile([128, T], f16, tag=f"m{h}", name=f"m{h}")
                nc.gpsimd.memset(m0[:], -60000.0)
                m_prev.append(m0)

            for t in range(NT):
                xt = []
                for k, xsrc in enumerate((x0, x1)):
                    if t == 0 and k == 0:
                        xk = x0_first
                        for z in range(1, Z):
                            nc.sync.dma_start(
                                out=xk[:, z, :],
                                in_=xsrc[t, :, z * T : (z + 1) * T],
                            )
                    elif t < 2:
                        # fine-grained first loads: the first matmul only waits
                        # for its z-slice, not the whole 393KB tile
                        xk = xpool.tile([128, Z, T], f16, tag=f"x{k}", name=f"xt{k}")
                        for z in range(Z):
                            nc.sync.dma_start(
                                out=xk[:, z, :],
                                in_=xsrc[t, :, z * T : (z + 1) * T],
                            )
                    else:
                        xk = xpool.tile([128, Z, T], f16, tag=f"x{k}", name=f"xt{k}")
                        nc.sync.dma_start(out=xk[:], in_=xsrc[t])
                    xt.append(xk)
                for h in range(H):
                    d = dpool.tile([128, Z, T], f32, name="d_psum")
                    # k-major order: 3 consecutive matmuls share the same
                    # stationary weights (one PSUM bank = 512 fp32 per mm).
                    for k in range(2):
                        for z in range(Z):
                            nc.tensor.matmul(
                                d[:, z, :],
                                wt_sb[k][:, h * 128 : (h + 1) * 128],
                                xt[k][:, z, :],
                                start=(k == 0),
                                stop=(k == 1),
                            )
                    dc = cpool.tile([128, Z, T], f16, tag="dc", name="dc")
                    nc.scalar.copy(dc[:], d[:])
                    # All-DVE chain: the tile scheduler's GpSimd cost model is
                    # ~1.8x optimistic, so any DVE<->Pool edge injects real-HW
                    # stalls into the static schedule. DVE 2x fp16 ops only.
                    # p slots: 0,1,2 = x*d per z; 3 = s1 = p0+p1; 4 = s = s1+p2.
                    # The fused add computes slabs {s1,s} = {p0,s1} + {p1,p2}
                    # in one instruction; slab 1 reads the s1 values slab 0
                    # wrote 512 elements earlier in the stream.
                    p = ppool.tile([128, 5, T], f16, tag="p", name="p")
                    nc.vector.tensor_tensor(p[:, 0:Z, :], xt[h][:], dc[:], op=mult)
                    nc.vector.tensor_tensor(
                        p[:, 3:5, :], p[:, 0:4:3, :], p[:, 1:3, :], op=add
                    )
                    m_new = mpool.tile([128, T], f16, tag=f"m{h}", name=f"m{h}n")
                    nc.vector.tensor_tensor(m_new[:], m_prev[h][:], p[:, 4, :], op=vmax)
                    m_prev[h] = m_new

            for h in range(H):
                vt = opool.tile([128, 8], f16, tag=f"v{h}", name=f"vt{h}")
                it = opool.tile([128, 8], u32, tag=f"i{h}", name=f"it{h}")
                nc.vector.max(vt[:], m_prev[h][:])
                nc.vector.max_index(it[:], vt[:], m_prev[h][:])
                nc.sync.dma_start(out=v8[h], in_=vt[:])
                nc.sync.dma_start(out=i8[h], in_=it[:])

    return nc


def _get_nc():
    if "nc" not in _cache:
        _cache["nc"] = _build_nc()
    return _cache["nc"]


def _make_in_maps(x, W):
    """Per-core input dict: fp16 tiled x halves + transposed fp16 W slices."""
    wt = np.ascontiguousarray(W.T).astype(np.float16)
    x16 = x.astype(np.float16)  # [B, C, Z, N]
    in_maps = []
    for b in range(B):
        m = {"wt0": wt[:128], "wt1": wt[128:]}
        for k in range(2):
            # [128, Z, NT, T] -> [NT, 128, Z*T]
            xk = x16[b, k * 128 : (k + 1) * 128].reshape(128, Z, NT, T)
            m[f"x{k}"] = np.ascontiguousarray(xk.transpose(2, 0, 1, 3)).reshape(
                NT, 128, ZT
            )
        in_maps.append(m)
    return in_maps


def _run_device(x, W):
    from concourse.bass_utils import run_bass_kernel_spmd

    nc = _get_nc()
    res = run_bass_kernel_spmd(nc, _make_in_maps(x, W), core_ids=list(range(B)))
    v8 = np.stack([r["v8"].reshape(C, 8) for r in res.results])  # [B, C, 8] f16
    i8 = np.stack([r["i8"].reshape(C, 8) for r in res.results])  # [B, C, 8] u32
    return v8, i8, res


def _host_finalize(x, W, i8):
    """Expand the 8 folded positions per row to 8*NT candidate indices,
    re-score them exactly in float64, and gather the winning 3-vector."""
    out = np.empty((B, C, Z), dtype=x.dtype)
    W64 = W.astype(np.float64)
    offs = (np.arange(NT, dtype=np.int64) * T)[None, :, None]  # [1, NT, 1]
    NC = NT * 8
    for b in range(B):
        xb = x[b]  # [C, Z, N] fp32
        q = np.minimum(i8[b].astype(np.int64), T - 1)  # [C, 8]
        I = (q[:, None, :] + offs).reshape(C, NC)  # [C, NC]
        xb64 = xb.astype(np.float64)
        s_cand = np.empty((C, NC), dtype=np.float64)
        blk = 64
        for r0 in range(0, C, blk):
            r1 = r0 + blk
            # cols[c_in, z, r, j] = x[b, c_in, z, I[r, j]]
            cols = xb64[:, :, I[r0:r1]]  # [C, Z, blk, NC]
            d_cand = np.einsum("rc,czrj->rzj", W64[r0:r1], cols)
            xr = np.take_along_axis(
                xb64[r0:r1], I[r0:r1, None, :], axis=2
            )  # [blk, Z, NC]
            s_cand[r0:r1] = (xr * d_cand).sum(axis=1)
        # argmax over candidates; break exact ties toward the smallest n
        # (matches jnp.argmax first-occurrence semantics).
        order = np.lexsort((I, -s_cand), axis=1)
        jbest = order[:, 0]
        nbest = I[np.arange(C), jbest]
        out[b] = np.take_along_axis(xb, nbest[:, None, None], axis=2)[:, :, 0]
    return out


def kernel(x, W):
    x = np.asarray(x, dtype=np.float32)
    W = np.asarray(W, dtype=np.float32)
    v8, i8, _ = _run_device(x, W)
    return _host_finalize(x, W, i8)


# revision 27
# speedup vs baseline: 1.0129x; 1.0129x over previous
"""Trainium2 Bass kernel for nn_MaxPool_730144440853.

Math (per batch b):
    d = einsum("czn,dc->dzn", x[b], W)
    scores[c, n] = sum_z x[b,c,z,n] * d[c,z,n]
    idx[c] = argmax_n scores[c, n]
    out[b, c, :] = x[b, c, :, idx[c]]

Sharding: data-parallel over batch B=8 across the 8 NeuronCores; W replicated.

Device pipeline (per core, fp16 inputs):
  - PE: d = W @ x per (n-tile, half, z), fp16 matmuls into fp32 PSUM, k-major
    so 3 consecutive matmuls share stationary weights.
  - Act: cast d PSUM fp32 -> SBUF fp16 (unlocks the DVE 2x 16-bit mode).
  - DVE (2x fp16): p = x * d, one fused strided-slab instruction for both
    z-adds, and a running elementwise fold m[q] = max_t s[t*512+q] across the
    16 n-tiles. Everything stays on DVE: cross-engine edges through Pool get
    mis-scheduled (the tile scheduler's GpSimd cost model is ~1.8x fast).
  - DVE tail: max8 + max_index over the folded [128, 512] max array only.

Device returns, per (b, c) row, the top-8 folded positions q_j. The true
argmax position n* = t*512+q satisfies q in {q_j} unless >=8 distinct folded
positions beat the true maximum within fp16 noise (~0.1% of sigma), which is
astronomically unlikely. Host expands the 8 q's to 8*16=128 candidate n's,
re-scores them exactly in float64 from the original fp32 inputs, and picks
the argmax (ties toward smallest n, matching jnp.argmax first-occurrence).
"""

import sys

sys.path.insert(0, "/opt/trn_rl_repo")

import numpy as np

B, C, Z, N = 8, 256, 3, 8192
H = C // 128  # partition halves (2)
T = 512  # n-tile width
NT = N // T
ZT = Z * T

_cache = {}


def _split_multiwait_bir(bir_json: bytes) -> bytes:
    """walrus in this toolchain rejects instructions carrying more than one
    semaphore wait ("Too many sync wait commands"). Rewrite the BIR so any
    instruction with >1 on_wait keeps only the last one; the others are
    hoisted into single-wait EventSemaphore instructions inserted just
    before it on the same engine (engine program order makes this
    equivalent)."""
    import json

    d = json.loads(bir_json)
    n_new = 0
    for fn in d.get("functions", []):
        for blk in fn.get("blocks", []):
            insts = blk.get("instructions", [])
            out = []
            for ins in insts:
                si = ins.get("sync_info")
                waits = si.get("on_wait") if si else None
                if waits and len(waits) > 1:
                    for w in waits[:-1]:
                        out.append(
                            {
                                "debug": ins.get("debug", 0),
                                "engine": ins["engine"],
                                "ins": [],
                                "name": f"{ins['name']}_hw{n_new}",
                                "opcode": "EventSemaphore",
                                "outs": [],
                                "sync_info": {"on_update": [], "on_wait": [w]},
                            }
                        )
                        n_new += 1
                    si["on_wait"] = [waits[-1]]
                out.append(ins)
            blk["instructions"] = out
    return json.dumps(d).encode()


def _apply_tile_patch():
    """Install the multi-wait splitter in front of walrus compilation."""
    from concourse import bass_utils, bass2jax

    if getattr(bass_utils, "_ant_split_multiwait", False):
        return

    orig = bass_utils.compile_bir_kernel

    def patched(bir_json, tmpdir, neff_name="file.neff"):
        return orig(_split_multiwait_bir(bir_json), tmpdir, neff_name=neff_name)

    bass_utils.compile_bir_kernel = patched
    bass2jax.compile_bir_kernel = patched
    bass_utils._ant_split_multiwait = True


def _build_nc():
    import concourse.bass as bass
    import concourse.mybir as mybir
    from concourse.tile import TileContext

    _apply_tile_patch()

    f16 = mybir.dt.float16
    f32 = mybir.dt.float32
    u32 = mybir.dt.uint32
    add = mybir.AluOpType.add
    mult = mybir.AluOpType.mult
    vmax = mybir.AluOpType.max

    nc = bass.Bass(target_bir_lowering=False)
    # x{k}[t] = fp16 tile [128, Z*T], channels k*128..k*128+127, n-tile t.
    x0 = nc.dram_tensor("x0", [NT, 128, ZT], f16, kind="ExternalInput")
    x1 = nc.dram_tensor("x1", [NT, 128, ZT], f16, kind="ExternalInput")
    # wt{k}[c_in - k*128, c_out] = W[c_out, c_in]; lhsT slices for the PE.
    wt0 = nc.dram_tensor("wt0", [128, C], f16, kind="ExternalInput")
    wt1 = nc.dram_tensor("wt1", [128, C], f16, kind="ExternalInput")
    v8 = nc.dram_tensor("v8", [H, 128, 8], f16, kind="ExternalOutput")
    i8 = nc.dram_tensor("i8", [H, 128, 8], u32, kind="ExternalOutput")

    with TileContext(nc) as tc:
        with (
            tc.tile_pool(name="wts", bufs=1) as wpool,
            tc.tile_pool(name="xin", bufs=6) as xpool,
            tc.tile_pool(name="dcast", bufs=6) as cpool,
            tc.tile_pool(name="prod", bufs=6) as ppool,
            tc.tile_pool(name="fold", bufs=2) as mpool,
            tc.tile_pool(name="psum", bufs=2, space="PSUM") as dpool,
            tc.tile_pool(name="outs", bufs=1) as opool,
        ):
            # First x chunk is dispatched before the W loads: W is only 64KB
            # and lands quickly, while the first matmul's x slice is the long
            # pole of the pipeline head.
            x0_first = xpool.tile([128, Z, T], f16, tag="x0", name="xt0_first")
            for half in range(2):
                nc.sync.dma_start(
                    out=x0_first[:, 0, half * (T // 2) : (half + 1) * (T // 2)],
                    in_=x0[0, :, half * (T // 2) : (half + 1) * (T // 2)],
                )

            wt_sb = []
            for k, wt in enumerate((wt0, wt1)):
                w = wpool.tile([128, C], f16, tag=f"wt{k}", name=f"wt_sb{k}")
                nc.sync.dma_start(out=w[:], in_=wt[:])
                wt_sb.append(w)

            # running folded max per half, ping-pong buffers via the pool
            # (memsets on the otherwise-idle Pool engine, off the DVE queue)
            m_prev = []
            for h in range(H):
                m0 = mpool.tile([128, T], f16, tag=f"m{h}", name=f"m{h}")
                nc.gpsimd.memset(m0[:], -60000.0)
                m_prev.append(m0)

            for t in range(NT):
                xt = []
                for k, xsrc in enumerate((x0, x1)):
                    if t == 0 and k == 0:
                        xk = x0_first
                        for z in range(1, Z):
                            nc.sync.dma_start(
                                out=xk[:, z, :],
                                in_=xsrc[t, :, z * T : (z + 1) * T],
                            )
                    elif t < 2:
                        # fine-grained first loads: the first matmul only waits
                        # for its z-slice, not the whole 393KB tile
                        xk = xpool.tile([128, Z, T], f16, tag=f"x{k}", name=f"xt{k}")
                        for z in range(Z):
                            nc.sync.dma_start(
                                out=xk[:, z, :],
                                in_=xsrc[t, :, z * T : (z + 1) * T],
                            )
                    else:
                        xk = xpool.tile([128, Z, T], f16, tag=f"x{k}", name=f"xt{k}")
                        nc.sync.dma_start(out=xk[:], in_=xsrc[t])
                    xt.append(xk)
                for h in range(H):
                    d = dpool.tile([128, Z, T], f32, name="d_psum")
                    dc = cpool.tile([128, Z, T], f16, tag="dc", name="dc")
                    p = ppool.tile([128, 5, T], f16, tag="p", name="p")
                    if t == 0:
                        # Pipeline-fill special case: z-major matmuls with
                        # per-z cast and multiply, so the first DVE op starts
                        # after 2 matmuls + a 512-el copy instead of a full
                        # 6-matmul group + 1536-el copy. The extra per-op
                        # overhead lands in otherwise-idle fill time.
                        for z in range(Z):
                            for k in range(2):
                                nc.tensor.matmul(
                                    d[:, z, :],
                                    wt_sb[k][:, h * 128 : (h + 1) * 128],
                                    xt[k][:, z, :],
                                    start=(k == 0),
                                    stop=(k == 1),
                                )
                            nc.scalar.copy(dc[:, z, :], d[:, z, :])
                            nc.vector.tensor_tensor(
                                p[:, z, :], xt[h][:, z, :], dc[:, z, :], op=mult
                            )
                    else:
                        # k-major order: 3 consecutive matmuls share the same
                        # stationary weights (one PSUM bank = 512 fp32 per mm).
                        for k in range(2):
                            for z in range(Z):
                                nc.tensor.matmul(
                                    d[:, z, :],
                                    wt_sb[k][:, h * 128 : (h + 1) * 128],
                                    xt[k][:, z, :],
                                    start=(k == 0),
                                    stop=(k == 1),
                                )
                        nc.scalar.copy(dc[:], d[:])
                        # All-DVE chain: the tile scheduler's GpSimd cost model
                        # is ~1.8x optimistic, so any DVE<->Pool edge injects
                        # real-HW stalls into the static schedule. DVE 2x fp16
                        # ops only. p slots: 0,1,2 = x*d per z; 3 = s1 = p0+p1;
                        # 4 = s = s1+p2. The fused add computes slabs
                        # {s1,s} = {p0,s1} + {p1,p2} in one instruction; slab 1
                        # reads the s1 values slab 0 wrote 512 elements earlier
                        # in the stream.
                        nc.vector.tensor_tensor(
                            p[:, 0:Z, :], xt[h][:], dc[:], op=mult
                        )
                    nc.vector.tensor_tensor(
                        p[:, 3:5, :], p[:, 0:4:3, :], p[:, 1:3, :], op=add
                    )
                    m_new = mpool.tile([128, T], f16, tag=f"m{h}", name=f"m{h}n")
                    nc.vector.tensor_tensor(m_new[:], m_prev[h][:], p[:, 4, :], op=vmax)
                    m_prev[h] = m_new

            for h in range(H):
                vt = opool.tile([128, 8], f16, tag=f"v{h}", name=f"vt{h}")
                it = opool.tile([128, 8], u32, tag=f"i{h}", name=f"it{h}")
                nc.vector.max(vt[:], m_prev[h][:])
                nc.vector.max_index(it[:], vt[:], m_prev[h][:])
                nc.sync.dma_start(out=v8[h], in_=vt[:])
                nc.sync.dma_start(out=i8[h], in_=it[:])

    return nc


def _get_nc():
    if "nc" not in _cache:
        _cache["nc"] = _build_nc()
    return _cache["nc"]


def _make_in_maps(x, W):
    """Per-core input dict: fp16 tiled x halves + transposed fp16 W slices."""
    wt = np.ascontiguousarray(W.T).astype(np.float16)
    x16 = x.astype(np.float16)  # [B, C, Z, N]
    in_maps = []
    for b in range(B):
        m = {"wt0": wt[:128], "wt1": wt[128:]}
        for k in range(2):
            # [128, Z, NT, T] -> [NT, 128, Z*T]
            xk = x16[b, k * 128 : (k + 1) * 128].reshape(128, Z, NT, T)
            m[f"x{k}"] = np.ascontiguousarray(xk.transpose(2, 0, 1, 3)).reshape(
                NT, 128, ZT
            )
        in_maps.append(m)
    return in_maps


def _run_device(x, W):
    from concourse.bass_utils import run_bass_kernel_spmd

    nc = _get_nc()
    res = run_bass_kernel_spmd(nc, _make_in_maps(x, W), core_ids=list(range(B)))
    v8 = np.stack([r["v8"].reshape(C, 8) for r in res.results])  # [B, C, 8] f16
    i8 = np.stack([r["i8"].reshape(C, 8) for r in res.results])  # [B, C, 8] u32
    return v8, i8, res


def _host_finalize(x, W, i8):
    """Expand the 8 folded positions per row to 8*NT candidate indices,
    re-score them exactly in float64, and gather the winning 3-vector."""
    out = np.empty((B, C, Z), dtype=x.dtype)
    W64 = W.astype(np.float64)
    offs = (np.arange(NT, dtype=np.int64) * T)[None, :, None]  # [1, NT, 1]
    NC = NT * 8
    for b in range(B):
        xb = x[b]  # [C, Z, N] fp32
        q = np.minimum(i8[b].astype(np.int64), T - 1)  # [C, 8]
        I = (q[:, None, :] + offs).reshape(C, NC)  # [C, NC]
        xb64 = xb.astype(np.float64)
        s_cand = np.empty((C, NC), dtype=np.float64)
        blk = 64
        for r0 in range(0, C, blk):
            r1 = r0 + blk
            # cols[c_in, z, r, j] = x[b, c_in, z, I[r, j]]
            cols = xb64[:, :, I[r0:r1]]  # [C, Z, blk, NC]
            d_cand = np.einsum("rc,czrj->rzj", W64[r0:r1], cols)
            xr = np.take_along_axis(
                xb64[r0:r1], I[r0:r1, None, :], axis=2
            )  # [blk, Z, NC]
            s_cand[r0:r1] = (xr * d_cand).sum(axis=1)
        # argmax over candidates; break exact ties toward the smallest n
        # (matches jnp.argmax first-occurrence semantics).
        order = np.lexsort((I, -s_cand), axis=1)
        jbest = order[:, 0]
        nbest = I[np.arange(C), jbest]
        out[b] = np.take_along_axis(xb, nbest[:, None, None], axis=2)[:, :, 0]
    return out


def kernel(x, W):
    x = np.asarray(x, dtype=np.float32)
    W = np.asarray(W, dtype=np.float32)
    v8, i8, _ = _run_device(x, W)
    return _host_finalize(x, W, i8)
